# revision 30
# baseline (speedup 1.0000x reference)
"""Multi-head self-attention Trainium2 kernel (8-core data parallel).

Reference computation (per batch b):
  q/k/v = einsum('sd,hda->hsa', x[b], W[:,i])       i in {0,1,2}
  scores = q @ k^T / sqrt(64); probs = softmax(scores)
  out = probs @ v; cat = concat heads [s, h*a]; z = cat @ Wz

Strategy per core (1 batch per core):
  - host pre-transposes x[b] -> xT [d, s] and flattens W head-major, all bf16
  - qT,kT computed W-stationary: qT[ha, s] tiles (2 heads per 128-partition tile)
  - v computed x-stationary in natural [t, ha] layout, stored interleaved with a
    ones column per head: v_sb[t, h, 0:64]=v, v_sb[t, h, 64]=1
  - scoresT[t, s] = kT.T @ qT per head (K=64); even/odd heads of a pair run in
    PE row-groups 0-1 / 2-3 concurrently (lhsT base partition 0 / 64)
  - exp on ScalarE with scale=1/8, no max subtraction (|scores/8| <~ 5.5)
  - out^T accumulated via lhsT=[v|1]: psum rows 0..63 = v^T @ expT (unnorm.),
    row 64 = sum_t expT = softmax denominator
  - normalize: reciprocal_approx_fast of row 64, gpsimd partition-broadcast,
    multiply -> catT[ha, s] bf16 (exactly the lhsT layout the final matmul wants)
  - zT[64, s] = Wz.T @ catT; PE-transpose to z[s, 64] fp32 and DMA out
"""

import sys
from contextlib import ExitStack

sys.path.insert(0, "/opt/trn_rl_repo")

import numpy as np
import ml_dtypes

import concourse.bass as bass
import concourse.bacc as bacc
import concourse.tile as tile
import concourse.mybir as mybir
from concourse.bass_utils import run_bass_kernel_spmd
from concourse.masks import make_identity

F32 = mybir.dt.float32
BF16 = mybir.dt.bfloat16
BF = ml_dtypes.bfloat16

S = 1024  # sequence length
D = 1024  # model dim
H = 16    # heads
A = 64    # attention dim per head
B = 8     # batch (one per core)
NT = 8    # 128-row tiles per 1024 dim

TRACE = False
LAST_EXEC_NS = None

_PROGRAM = None


def _build_program():
    nc = bacc.Bacc("TRN2", target_bir_lowering=False, debug=False)

    xT = nc.dram_tensor("xT", [D, S], BF16, kind="ExternalInput").ap()
    wq = nc.dram_tensor("wq", [D, H * A], BF16, kind="ExternalInput").ap()
    wk = nc.dram_tensor("wk", [D, H * A], BF16, kind="ExternalInput").ap()
    wv = nc.dram_tensor("wv", [D, H * A], BF16, kind="ExternalInput").ap()
    wz = nc.dram_tensor("wz", [H * A, A], BF16, kind="ExternalInput").ap()
    out = nc.dram_tensor("out", [S, A], F32, kind="ExternalOutput").ap()

    with tile.TileContext(nc) as tc:
        with (
            tc.tile_pool(name="persist", bufs=1) as pers,
            tc.tile_pool(name="probs", bufs=23) as ppool,
            tc.tile_pool(name="small", bufs=4) as small,
            tc.tile_pool(name="pssc", bufs=2, space="PSUM") as pssc,
            tc.tile_pool(name="pspv", bufs=2, space="PSUM") as pspv,
            tc.tile_pool(name="psqk", bufs=2, space="PSUM") as psqk,
        ):
            wz_sb = pers.tile([128, NT, A], BF16)
            v_sb = pers.tile([128, NT, H, A + 1], BF16)
            qt_sb = pers.tile([128, NT, S], BF16)
            kt_sb = pers.tile([128, NT, S], BF16)
            catt_sb = pers.tile([128, NT, S], BF16)
            ident = pers.tile([64, 64], F32)
            zt_sb = pers.tile([64, S], F32)
            out_sb = pers.tile([128, NT, A], F32)

            qkstack = ExitStack()
            wqkp = qkstack.enter_context(tc.tile_pool(name="wqkp", bufs=1))
            wstack = ExitStack()
            wvp = wstack.enter_context(tc.tile_pool(name="wvp", bufs=1))

            wv_sb = wvp.tile([128, NT, H * A], BF16)
            xt_sb = wqkp.tile([128, NT, S], BF16)
            wq_sb = wqkp.tile([128, NT, H * A], BF16)
            wk_sb = wqkp.tile([128, NT, H * A], BF16)

            # input DMAs: x+wv first (v phase), then wq/wk, then wz
            for d in range(NT):
                r = slice(d * 128, (d + 1) * 128)
                nc.sync.dma_start(out=wv_sb[:, d, :], in_=wv[r, :])
                nc.sync.dma_start(out=xt_sb[:, d, 0:512], in_=xT[r, 0:512])
            for d in range(NT):
                r = slice(d * 128, (d + 1) * 128)
                nc.sync.dma_start(out=xt_sb[:, d, 512:1024], in_=xT[r, 512:1024])
            for d in range(NT):
                r = slice(d * 128, (d + 1) * 128)
                nc.sync.dma_start(out=wq_sb[:, d, :], in_=wq[r, :])
                nc.sync.dma_start(out=wk_sb[:, d, :], in_=wk[r, :])
                nc.sync.dma_start(out=wz_sb[:, d, :], in_=wz[r, :])

            # ones column per head for the softmax denominator row
            nc.vector.memset(v_sb[:, :, :, A : A + 1], 1.0)
            make_identity(nc, ident)

            # warmup burst: dense dummy matmuls at t=0 lift the PE HAM clock
            # gate to 8/8 before the DMA-paced V phase begins
            warm_sb = pers.tile([128, 256], BF16)
            nc.gpsimd.memset(warm_sb[:], 0.0)
            _wid = [0]

            def keep_warm(n):
                # dummy matmuls with no data deps: the scheduler slots them
                # into PE-idle stretches, keeping the HAM clock gate at 8/8
                _wid[0] += 1
                pw = pssc.tile([128, 1024], F32, tag="sc", name=f"warm_{_wid[0]}")
                for _ in range(n):
                    nc.tensor.matmul(
                        pw[:, 0:256], warm_sb[:, 0:128], warm_sb[:], start=True, stop=True
                    )

            keep_warm(20)

            # ---- V (natural [t, ha] layout, x-stationary) ----
            for tt in range(NT):
                for nh in range(2):
                    pv = psqk.tile([128, 512], F32, tag="qk")
                    for d in range(NT):
                        nc.tensor.matmul(
                            pv[:],
                            xt_sb[:, d, tt * 128 : (tt + 1) * 128],
                            wv_sb[:, d, nh * 512 : (nh + 1) * 512],
                            start=(d == 0),
                            stop=(d == NT - 1),
                        )
                    nc.vector.tensor_copy(
                        out=v_sb[:, tt, nh * 8 : (nh + 1) * 8, 0:A],
                        in_=pv[:].rearrange("p (h a) -> p h a", h=8),
                    )
                if tt < 3:
                    keep_warm(8 - 2 * tt)
            wstack.close()  # frees wv_sb

            # ---- fused per head-pair: Q^T/K^T projection, then attention ----
            for hp in range(NT):
                for w_sb, dst in ((wq_sb, qt_sb), (wk_sb, kt_sb)):
                    for sh in range(2):
                        pq = psqk.tile([128, 512], F32, tag="qk")
                        ssl = slice(sh * 512, (sh + 1) * 512)
                        for d in range(NT):
                            nc.tensor.matmul(
                                pq[:],
                                w_sb[:, d, hp * 128 : (hp + 1) * 128],
                                xt_sb[:, d, ssl],
                                start=(d == 0),
                                stop=(d == NT - 1),
                            )
                        nc.vector.tensor_copy(out=dst[:, hp, ssl], in_=pq[:])

                probs = [
                    [
                        ppool.tile(
                            [128, 2, 512], BF16, tag="probs", name=f"probs_{hp}_{tt}_{sh}"
                        )
                        for sh in range(2)
                    ]
                    for tt in range(NT)
                ]
                def scores_exp(tt, sh):
                    ssl = slice(sh * 512, (sh + 1) * 512)
                    ps = pssc.tile([128, 1024], F32, tag="sc", name=f"ps_{hp}_{tt}_{sh}")
                    for par in range(2):
                        po = par * 64
                        nc.tensor.matmul(
                            ps[:, par * 512 : (par + 1) * 512],
                            kt_sb[po : po + 64, hp, tt * 128 : (tt + 1) * 128],
                            qt_sb[po : po + 64, hp, ssl],
                            start=True,
                            stop=True,
                        )
                    nc.scalar.activation(
                        out=probs[tt][sh][:],
                        in_=ps[:].rearrange("p (a b) -> p a b", a=2),
                        func=mybir.ActivationFunctionType.Exp,
                        scale=0.125,
                    )

                def normalize(par, sh, po_ps):
                    po = par * 64
                    ssl = slice(sh * 512, (sh + 1) * 512)
                    den = small.tile([1, 512], F32, tag="den", name=f"den_{hp}_{par}_{sh}")
                    nc.vector.tensor_copy(out=den[:], in_=po_ps[A : A + 1, :])
                    recip = small.tile([1, 512], F32, tag="recip", name=f"rc_{hp}_{par}_{sh}")
                    nc.vector.reciprocal_approx_fast(out=recip[:], in_=den[:])
                    bc = small.tile([64, 512], F32, tag="bc", name=f"bc_{hp}_{par}_{sh}")
                    nc.gpsimd.partition_broadcast(bc[:], recip[:])
                    nc.vector.tensor_mul(
                        catt_sb[po : po + 64, hp, ssl], po_ps[0:A, :], bc[:]
                    )

                if hp < NT - 1:
                    for tt in range(NT):
                        for sh in range(2):
                            scores_exp(tt, sh)
                    for sh in range(2):
                        for par in range(2):
                            h = 2 * hp + par
                            po_ps = pspv.tile(
                                [A + 1, 512], F32, tag="pv", name=f"pv_{h}_{sh}"
                            )
                            for tt in range(NT):
                                nc.tensor.matmul(
                                    po_ps[:],
                                    v_sb[:, tt, h, :],
                                    probs[tt][sh][:, par, :],
                                    start=(tt == 0),
                                    stop=(tt == NT - 1),
                                )
                            normalize(par, sh, po_ps)
                else:
                    # last pair: interleave PV with scores/exp per t-tile so the
                    # PE keeps work during the exp-paced pipeline drain
                    for sh in range(2):
                        pvt = [
                            pspv.tile([A + 1, 512], F32, tag="pv", name=f"pvL_{par}_{sh}")
                            for par in range(2)
                        ]
                        for tt in range(NT):
                            scores_exp(tt, sh)
                            for par in range(2):
                                nc.tensor.matmul(
                                    pvt[par][:],
                                    v_sb[:, tt, 2 * hp + par, :],
                                    probs[tt][sh][:, par, :],
                                    start=(tt == 0),
                                    stop=(tt == NT - 1),
                                )
                        for par in range(2):
                            normalize(par, sh, pvt[par])
            qkstack.close()  # frees xt/wq/wk
            keep_warm(14)

            # ---- final projection z^T = Wz^T @ catT, split by s-half ----
            for sh in range(2):
                ssl = slice(sh * 512, (sh + 1) * 512)
                pz = psqk.tile([128, 512], F32, tag="qk", name=f"pz_{sh}")
                for kt in range(NT):
                    nc.tensor.matmul(
                        pz[0:A, :],
                        wz_sb[:, kt, :],
                        catt_sb[:, kt, ssl],
                        start=(kt == 0),
                        stop=(kt == NT - 1),
                    )
                nc.scalar.copy(out=zt_sb[:, ssl], in_=pz[0:A, :])
                # transpose zT [64, s] -> z [s, 64] via PE, 128 rows at a time
                for st in range(4 * sh, 4 * (sh + 1)):
                    pt = psqk.tile([128, 512], F32, tag="qk", name=f"pt_{st}")
                    nc.tensor.transpose(
                        pt[:, 0:A], zt_sb[:, st * 128 : (st + 1) * 128], ident[:]
                    )
                    nc.scalar.copy(out=out_sb[:, st, :], in_=pt[:, 0:A])

                    nc.sync.dma_start(
                        out=out.rearrange("(st p) n -> p st n", p=128)[:, st, :],
                        in_=out_sb[:, st, :],
                    )

    nc.compile()
    return nc


def _get_program():
    global _PROGRAM
    if _PROGRAM is None:
        _PROGRAM = _build_program()
    return _PROGRAM


def kernel(x: np.ndarray, W: np.ndarray, Wz: np.ndarray) -> np.ndarray:
    global LAST_EXEC_NS
    x = np.asarray(x, dtype=np.float32)
    W = np.asarray(W, dtype=np.float32)
    Wz = np.asarray(Wz, dtype=np.float32)
    assert x.shape == (B, S, D) and W.shape == (H, 3, D, A) and Wz.shape == (H * A, A)

    # host-side prep: flatten weights head-major [d, h*a], cast to bf16
    Wf = W.astype(BF)
    wq_h = np.ascontiguousarray(Wf[:, 0].transpose(1, 0, 2).reshape(D, H * A))
    wk_h = np.ascontiguousarray(Wf[:, 1].transpose(1, 0, 2).reshape(D, H * A))
    wv_h = np.ascontiguousarray(Wf[:, 2].transpose(1, 0, 2).reshape(D, H * A))
    wz_h = np.ascontiguousarray(Wz.astype(BF))

    in_maps = []
    for b in range(B):
        xt = np.ascontiguousarray(x[b].T.astype(BF))
        in_maps.append({"xT": xt, "wq": wq_h, "wk": wk_h, "wv": wv_h, "wz": wz_h})

    nc = _get_program()
    last_exc = None
    for attempt in range(3):
        try:
            res = run_bass_kernel_spmd(nc, in_maps, core_ids=list(range(B)), trace=TRACE)
            break
        except Exception as e:  # transient device faults (e.g. NRT unrecoverable)
            last_exc = e
            import time

            time.sleep(2.0)
    else:
        raise last_exc
    LAST_EXEC_NS = res.exec_time_ns
    return np.stack([res.results[b]["out"] for b in range(B)], axis=0)


# revision 31
# speedup vs baseline: 1.0074x; 1.0074x over previous
"""Multi-head self-attention Trainium2 kernel (8-core data parallel).

Reference computation (per batch b):
  q/k/v = einsum('sd,hda->hsa', x[b], W[:,i])       i in {0,1,2}
  scores = q @ k^T / sqrt(64); probs = softmax(scores)
  out = probs @ v; cat = concat heads [s, h*a]; z = cat @ Wz

Strategy per core (1 batch per core):
  - host pre-transposes x[b] -> xT [d, s] and flattens W head-major, all bf16
  - qT,kT computed W-stationary: qT[ha, s] tiles (2 heads per 128-partition tile)
  - v computed x-stationary in natural [t, ha] layout, stored interleaved with a
    ones column per head: v_sb[t, h, 0:64]=v, v_sb[t, h, 64]=1
  - scoresT[t, s] = kT.T @ qT per head (K=64); even/odd heads of a pair run in
    PE row-groups 0-1 / 2-3 concurrently (lhsT base partition 0 / 64)
  - exp on ScalarE with scale=1/8, no max subtraction (|scores/8| <~ 5.5)
  - out^T accumulated via lhsT=[v|1]: psum rows 0..63 = v^T @ expT (unnorm.),
    row 64 = sum_t expT = softmax denominator
  - normalize: reciprocal_approx_fast of row 64, gpsimd partition-broadcast,
    multiply -> catT[ha, s] bf16 (exactly the lhsT layout the final matmul wants)
  - zT[64, s] = Wz.T @ catT; PE-transpose to z[s, 64] fp32 and DMA out
"""

import sys
from contextlib import ExitStack

sys.path.insert(0, "/opt/trn_rl_repo")

import numpy as np
import ml_dtypes

import concourse.bass as bass
import concourse.bacc as bacc
import concourse.tile as tile
import concourse.mybir as mybir
from concourse.bass_utils import run_bass_kernel_spmd
from concourse.masks import make_identity

F32 = mybir.dt.float32
BF16 = mybir.dt.bfloat16
BF = ml_dtypes.bfloat16

S = 1024  # sequence length
D = 1024  # model dim
H = 16    # heads
A = 64    # attention dim per head
B = 8     # batch (one per core)
NT = 8    # 128-row tiles per 1024 dim

TRACE = False
LAST_EXEC_NS = None

_PROGRAM = None


def _build_program():
    nc = bacc.Bacc("TRN2", target_bir_lowering=False, debug=False)

    xT = nc.dram_tensor("xT", [D, S], BF16, kind="ExternalInput").ap()
    wq = nc.dram_tensor("wq", [D, H * A], BF16, kind="ExternalInput").ap()
    wk = nc.dram_tensor("wk", [D, H * A], BF16, kind="ExternalInput").ap()
    wv = nc.dram_tensor("wv", [D, H * A], BF16, kind="ExternalInput").ap()
    wz = nc.dram_tensor("wz", [H * A, A], BF16, kind="ExternalInput").ap()
    out = nc.dram_tensor("out", [S, A], F32, kind="ExternalOutput").ap()

    with tile.TileContext(nc) as tc:
        with (
            tc.tile_pool(name="persist", bufs=1) as pers,
            tc.tile_pool(name="probs", bufs=23) as ppool,
            tc.tile_pool(name="small", bufs=4) as small,
            tc.tile_pool(name="pssc", bufs=2, space="PSUM") as pssc,
            tc.tile_pool(name="pspv", bufs=2, space="PSUM") as pspv,
            tc.tile_pool(name="psqk", bufs=2, space="PSUM") as psqk,
        ):
            wz_sb = pers.tile([128, NT, A], BF16)
            v_sb = pers.tile([128, NT, H, A + 1], BF16)
            qt_sb = pers.tile([128, NT, S], BF16)
            kt_sb = pers.tile([128, NT, S], BF16)
            catt_sb = pers.tile([128, NT, S], BF16)
            ident = pers.tile([64, 64], F32)
            zt_sb = pers.tile([64, S], F32)
            out_sb = pers.tile([128, NT, A], F32)

            qkstack = ExitStack()
            wqkp = qkstack.enter_context(tc.tile_pool(name="wqkp", bufs=1))
            wstack = ExitStack()
            wvp = wstack.enter_context(tc.tile_pool(name="wvp", bufs=1))

            wv_sb = wvp.tile([128, NT, H * A], BF16)
            xt_sb = wqkp.tile([128, NT, S], BF16)
            wq_sb = wqkp.tile([128, NT, H * A], BF16)
            wk_sb = wqkp.tile([128, NT, H * A], BF16)

            # input DMAs: x+wv first (v phase), then wq/wk, then wz
            for d in range(NT):
                r = slice(d * 128, (d + 1) * 128)
                nc.sync.dma_start(out=wv_sb[:, d, :], in_=wv[r, :])
                nc.sync.dma_start(out=xt_sb[:, d, 0:512], in_=xT[r, 0:512])
            for d in range(NT):
                r = slice(d * 128, (d + 1) * 128)
                nc.sync.dma_start(out=xt_sb[:, d, 512:1024], in_=xT[r, 512:1024])
            for d in range(NT):
                r = slice(d * 128, (d + 1) * 128)
                nc.sync.dma_start(out=wq_sb[:, d, :], in_=wq[r, :])
                nc.sync.dma_start(out=wk_sb[:, d, :], in_=wk[r, :])
                nc.sync.dma_start(out=wz_sb[:, d, :], in_=wz[r, :])

            # ones column per head for the softmax denominator row
            nc.vector.memset(v_sb[:, :, :, A : A + 1], 1.0)
            make_identity(nc, ident)

            # warmup burst: dense dummy matmuls at t=0 lift the PE HAM clock
            # gate to 8/8 before the DMA-paced V phase begins
            warm_sb = pers.tile([128, 256], BF16)
            nc.gpsimd.memset(warm_sb[:], 0.0)
            _wid = [0]

            def keep_warm(n):
                # dummy matmuls with no data deps: the scheduler slots them
                # into PE-idle stretches, keeping the HAM clock gate at 8/8
                _wid[0] += 1
                pw = pssc.tile([128, 1024], F32, tag="sc", name=f"warm_{_wid[0]}")
                for _ in range(n):
                    nc.tensor.matmul(
                        pw[:, 0:256], warm_sb[:, 0:128], warm_sb[:], start=True, stop=True
                    )

            keep_warm(20)

            # ---- V (natural [t, ha] layout, x-stationary) ----
            for tt in range(NT):
                for nh in range(2):
                    pv = psqk.tile([128, 512], F32, tag="qk")
                    for d in range(NT):
                        nc.tensor.matmul(
                            pv[:],
                            xt_sb[:, d, tt * 128 : (tt + 1) * 128],
                            wv_sb[:, d, nh * 512 : (nh + 1) * 512],
                            start=(d == 0),
                            stop=(d == NT - 1),
                        )
                    nc.vector.tensor_copy(
                        out=v_sb[:, tt, nh * 8 : (nh + 1) * 8, 0:A],
                        in_=pv[:].rearrange("p (h a) -> p h a", h=8),
                    )
                if tt < 3:
                    keep_warm(8 - 2 * tt)
            wstack.close()  # frees wv_sb

            # ---- fused per head-pair: Q^T/K^T projection, then attention ----
            for hp in range(NT):
                for w_sb, dst in ((wq_sb, qt_sb), (wk_sb, kt_sb)):
                    for sh in range(2):
                        pq = psqk.tile([128, 512], F32, tag="qk")
                        ssl = slice(sh * 512, (sh + 1) * 512)
                        for d in range(NT):
                            nc.tensor.matmul(
                                pq[:],
                                w_sb[:, d, hp * 128 : (hp + 1) * 128],
                                xt_sb[:, d, ssl],
                                start=(d == 0),
                                stop=(d == NT - 1),
                            )
                        nc.vector.tensor_copy(out=dst[:, hp, ssl], in_=pq[:])

                probs = [
                    [
                        ppool.tile(
                            [128, 2, 512], BF16, tag="probs", name=f"probs_{hp}_{tt}_{sh}"
                        )
                        for sh in range(2)
                    ]
                    for tt in range(NT)
                ]
                def scores_exp(tt, sh):
                    ssl = slice(sh * 512, (sh + 1) * 512)
                    ps = pssc.tile([128, 1024], F32, tag="sc", name=f"ps_{hp}_{tt}_{sh}")
                    for par in range(2):
                        po = par * 64
                        nc.tensor.matmul(
                            ps[:, par * 512 : (par + 1) * 512],
                            kt_sb[po : po + 64, hp, tt * 128 : (tt + 1) * 128],
                            qt_sb[po : po + 64, hp, ssl],
                            start=True,
                            stop=True,
                        )
                    nc.scalar.activation(
                        out=probs[tt][sh][:],
                        in_=ps[:].rearrange("p (a b) -> p a b", a=2),
                        func=mybir.ActivationFunctionType.Exp,
                        scale=0.125,
                    )

                def normalize(par, sh, po_ps):
                    po = par * 64
                    ssl = slice(sh * 512, (sh + 1) * 512)
                    den = small.tile([1, 512], F32, tag="den", name=f"den_{hp}_{par}_{sh}")
                    nc.vector.tensor_copy(out=den[:], in_=po_ps[A : A + 1, :])
                    recip = small.tile([1, 512], F32, tag="recip", name=f"rc_{hp}_{par}_{sh}")
                    nc.vector.reciprocal_approx_fast(out=recip[:], in_=den[:])
                    bc = small.tile([64, 512], F32, tag="bc", name=f"bc_{hp}_{par}_{sh}")
                    nc.gpsimd.partition_broadcast(bc[:], recip[:])
                    nc.vector.tensor_mul(
                        catt_sb[po : po + 64, hp, ssl], po_ps[0:A, :], bc[:]
                    )

                if hp < NT - 1:
                    for tt in range(NT):
                        for sh in range(2):
                            scores_exp(tt, sh)
                    for sh in range(2):
                        for par in range(2):
                            h = 2 * hp + par
                            po_ps = pspv.tile(
                                [A + 1, 512], F32, tag="pv", name=f"pv_{h}_{sh}"
                            )
                            for tt in range(NT):
                                nc.tensor.matmul(
                                    po_ps[:],
                                    v_sb[:, tt, h, :],
                                    probs[tt][sh][:, par, :],
                                    start=(tt == 0),
                                    stop=(tt == NT - 1),
                                )
                            normalize(par, sh, po_ps)
                else:
                    # last pair: interleave PV with scores/exp per t-tile so the
                    # PE keeps work during the exp-paced pipeline drain
                    for sh in range(2):
                        pvt = [
                            pspv.tile([A + 1, 512], F32, tag="pv", name=f"pvL_{par}_{sh}")
                            for par in range(2)
                        ]
                        for tt in range(NT):
                            scores_exp(tt, sh)
                            for par in range(2):
                                nc.tensor.matmul(
                                    pvt[par][:],
                                    v_sb[:, tt, 2 * hp + par, :],
                                    probs[tt][sh][:, par, :],
                                    start=(tt == 0),
                                    stop=(tt == NT - 1),
                                )
                        for par in range(2):
                            normalize(par, sh, pvt[par])
            qkstack.close()  # frees xt/wq/wk
            keep_warm(14)

            # ---- final projection z^T = Wz^T @ catT, split by s-half ----
            for sh in range(2):
                ssl = slice(sh * 512, (sh + 1) * 512)
                pz = psqk.tile([128, 512], F32, tag="qk", name=f"pz_{sh}")
                for kt in range(NT):
                    nc.tensor.matmul(
                        pz[0:A, :],
                        wz_sb[:, kt, :],
                        catt_sb[:, kt, ssl],
                        start=(kt == 0),
                        stop=(kt == NT - 1),
                    )
                nc.vector.tensor_copy(out=zt_sb[:, ssl], in_=pz[0:A, :])
                # transpose zT [64, s] -> z [s, 64] via PE, 128 rows at a time
                for st in range(4 * sh, 4 * (sh + 1)):
                    pt = psqk.tile([128, 512], F32, tag="qk", name=f"pt_{st}")
                    nc.tensor.transpose(
                        pt[:, 0:A], zt_sb[:, st * 128 : (st + 1) * 128], ident[:]
                    )
                    nc.vector.tensor_copy(out=out_sb[:, st, :], in_=pt[:, 0:A])

                    nc.sync.dma_start(
                        out=out.rearrange("(st p) n -> p st n", p=128)[:, st, :],
                        in_=out_sb[:, st, :],
                    )

    nc.compile()
    return nc


def _get_program():
    global _PROGRAM
    if _PROGRAM is None:
        _PROGRAM = _build_program()
    return _PROGRAM


def kernel(x: np.ndarray, W: np.ndarray, Wz: np.ndarray) -> np.ndarray:
    global LAST_EXEC_NS
    x = np.asarray(x, dtype=np.float32)
    W = np.asarray(W, dtype=np.float32)
    Wz = np.asarray(Wz, dtype=np.float32)
    assert x.shape == (B, S, D) and W.shape == (H, 3, D, A) and Wz.shape == (H * A, A)

    # host-side prep: flatten weights head-major [d, h*a], cast to bf16
    Wf = W.astype(BF)
    wq_h = np.ascontiguousarray(Wf[:, 0].transpose(1, 0, 2).reshape(D, H * A))
    wk_h = np.ascontiguousarray(Wf[:, 1].transpose(1, 0, 2).reshape(D, H * A))
    wv_h = np.ascontiguousarray(Wf[:, 2].transpose(1, 0, 2).reshape(D, H * A))
    wz_h = np.ascontiguousarray(Wz.astype(BF))

    in_maps = []
    for b in range(B):
        xt = np.ascontiguousarray(x[b].T.astype(BF))
        in_maps.append({"xT": xt, "wq": wq_h, "wk": wk_h, "wv": wv_h, "wz": wz_h})

    nc = _get_program()
    last_exc = None
    for attempt in range(3):
        try:
            res = run_bass_kernel_spmd(nc, in_maps, core_ids=list(range(B)), trace=TRACE)
            break
        except Exception as e:  # transient device faults (e.g. NRT unrecoverable)
            last_exc = e
            import time

            time.sleep(2.0)
    else:
        raise last_exc
    LAST_EXEC_NS = res.exec_time_ns
    return np.stack([res.results[b]["out"] for b in range(B)], axis=0)
